# revision 16
# baseline (speedup 1.0000x reference)
"""DWFGCN (GCN + TSK fuzzy layers) Trainium2 Bass kernel, 8-core SPMD.

Strategy (graph/data parallel, per sharding hint):
  - Nodes are relabeled and partitioned across the 8 cores (destination
    sharding).  Each core owns ~N/8 destination nodes, grouped into 64-wide
    "blocks"; per-(block, src-window) edge segments are rank-matched across
    cores so one compiled NEFF serves all 8 cores.
  - GCN aggregation commutes with the linear transform, so layer 1 gathers
    raw x rows (512B) with `dma_gather`, folds the edge norm into a one-hot
    "assignment" matrix built in one DVE op, and segment-sums on the PE via
    PSUM-accumulating matmuls.  The W1 transform, TSK fuzzify 1, and the W2
    transform (pre-scaled by dinv2[src]) run per 512-destination chunk,
    producing the layer-2 gather table.
  - Kernel 2 repeats the propagate with the (replicated) layer-2 table and
    runs fuzzify 2.  The host relays the table between dispatches (the
    all-gather of the sharding hint).
"""

import math
from contextlib import ExitStack

import numpy as np

from concourse import bass, mybir, bacc, tile, library_config
from concourse.bass_utils import run_bass_kernel_spmd

F32 = mybir.dt.float32
F16 = mybir.dt.float16
F32R = mybir.dt.float32r
I16 = mybir.dt.int16
I32 = mybir.dt.int32

NCORES = 8
BLK = 64              # dst-block width (matmul N window)
CHUNK_BLOCKS = 8      # dst blocks per transform chunk (512 dsts)
NG = 4                # src index windows (int16 offset range)
P = 128


def _ceil_to(x, m):
    return (x + m - 1) // m * m


# ---------------------------------------------------------------------------
# Host preprocessing: relabel nodes, build the (cross-core uniform) schedule
# and the per-core edge-slot arrays.
# ---------------------------------------------------------------------------

class Plan:
    pass


def preprocess(x, edge_index, edge_weight, R):
    N, D = x.shape
    E = edge_index.shape[1]
    src = np.asarray(edge_index[0], np.int64)
    dst = np.asarray(edge_index[1], np.int64)
    w = np.asarray(edge_weight, np.float32)

    npc = _ceil_to(_ceil_to(N, NCORES) // NCORES, BLK)   # padded nodes per core
    NPAD = npc * NCORES
    WPAD = NPAD // NG
    assert WPAD <= 32768, WPAD
    assert (2 * npc) == WPAD, (npc, WPAD)  # window == 2 cores
    nblocks = npc // BLK

    # degrees / normalization (self-loop weight 1 included)
    deg1 = np.bincount(dst, weights=w.astype(np.float64), minlength=N) + 1.0
    dinv1 = (1.0 / np.sqrt(deg1)).astype(np.float32)
    indeg = np.bincount(dst, minlength=N)
    deg2 = indeg + 1.0
    dinv2 = (1.0 / np.sqrt(deg2)).astype(np.float32)

    # ---- phase 1: nodes -> cores (snake deal by in-degree) ----
    order = np.argsort(-indeg, kind="stable")
    snake = np.concatenate([np.arange(NCORES), np.arange(NCORES - 1, -1, -1)])
    core_of = np.empty(N, np.int64)
    core_of[order] = np.resize(snake, N)
    gid_of = core_of // 2                      # src window of each node

    # per-(node, g) incoming counts (incl. self edge)
    eg = gid_of[src]
    cnt = np.bincount(dst * NG + eg, minlength=N * NG).reshape(N, NG)
    cnt[np.arange(N), gid_of] += 1
    tot = cnt.sum(1)

    # ---- phase 2: per core, deal nodes into blocks, rank blocks ----
    rid_of = np.full(N, -1, np.int64)
    orig_of = np.full(NPAD, -1, np.int64)
    counts = np.zeros((NCORES, nblocks, NG), np.int64)
    bsnake = np.concatenate([np.arange(nblocks), np.arange(nblocks - 1, -1, -1)])
    for c in range(NCORES):
        nodes = np.where(core_of == c)[0]
        o = nodes[np.argsort(-tot[nodes], kind="stable")]
        blk_of = np.resize(bsnake, len(o))
        cc = np.zeros((nblocks, NG), np.int64)
        for g in range(NG):
            cc[:, g] = np.bincount(blk_of, weights=cnt[o, g], minlength=nblocks)
        rank_order = np.argsort(-cc.sum(1), kind="stable")   # rank -> block
        rank_of_block = np.empty(nblocks, np.int64)
        rank_of_block[rank_order] = np.arange(nblocks)
        counts[c] = cc[rank_order]
        # slot within block = occurrence number in deal order
        pos = np.argsort(blk_of, kind="stable")
        sb = blk_of[pos]
        occ = np.arange(len(o)) - np.searchsorted(sb, sb)
        slot = np.empty(len(o), np.int64)
        slot[pos] = occ
        assert slot.max() < BLK
        rid = c * npc + rank_of_block[blk_of] * BLK + slot
        rid_of[o] = rid
        orig_of[rid] = o

    padded = counts.max(axis=0)                               # [nblocks, NG]

    # ---- schedule: chunks of CHUNK_BLOCKS ranks; per (chunk,g) one call ----
    nchunks = math.ceil(nblocks / CHUNK_BLOCKS)
    seg_start = np.zeros((nblocks, NG), np.int64)   # slot offset of each segment
    call_slots = np.zeros((nchunks, NG), np.int64)
    call_start = np.zeros((nchunks, NG), np.int64)
    pos = 0
    for ch in range(nchunks):
        r0, r1 = ch * CHUNK_BLOCKS, min((ch + 1) * CHUNK_BLOCKS, nblocks)
        for g in range(NG):
            call_start[ch, g] = pos
            for r in range(r0, r1):
                seg_start[r, g] = pos
                pos += padded[r, g]
            pos = _ceil_to(pos - call_start[ch, g], P) + call_start[ch, g]
            call_slots[ch, g] = pos - call_start[ch, g]
    TS = pos                                         # total slots per core

    # ---- per-core slot arrays ----
    rsrc_r = rid_of[src]
    rdst_r = rid_of[dst]
    rsrc = np.concatenate([rsrc_r, rid_of])          # + self edges
    rdst = np.concatenate([rdst_r, rid_of])
    n1 = np.concatenate([w * dinv1[src], dinv1])     # L1 per-edge scalar
    ecore = rdst // npc
    erank = (rdst % npc) // BLK
    # dstl is chunk-relative: (rank % CHUNK_BLOCKS)*BLK + slot, in [0, 512)
    _rank_all = (rdst % npc) // BLK
    edstl = ((_rank_all % CHUNK_BLOCKS) * BLK + rdst % BLK).astype(np.float32)
    egrp = rsrc // WPAD
    okey = np.lexsort((rsrc, egrp, erank, ecore))
    rsrc, rdst, n1 = rsrc[okey], rdst[okey], n1[okey]
    ecore, erank, edstl, egrp = ecore[okey], erank[okey], edstl[okey], egrp[okey]
    # position within own (core, rank, g) segment
    segkey = (ecore * nblocks + erank) * NG + egrp
    occ = np.arange(len(segkey)) - np.searchsorted(segkey, segkey)
    spos = seg_start[erank, egrp] + occ

    IDX = np.zeros((NCORES, TS), np.int16)
    DSTL = np.full((NCORES, TS), -1.0, np.float32)
    NORM1 = np.zeros((NCORES, TS), np.float32)
    NORM2 = np.zeros((NCORES, TS), np.float32)
    IDX[ecore, spos] = (rsrc - egrp * WPAD).astype(np.int16)
    DSTL[ecore, spos] = edstl
    NORM1[ecore, spos] = n1
    NORM2[ecore, spos] = 1.0

    # wrapped layouts (per call)
    def wrap_calls(arr, p):
        outs = []
        for ch in range(nchunks):
            for g in range(NG):
                s0 = call_start[ch, g]
                s1 = s0 + call_slots[ch, g]
                blkv = arr[:, s0:s1].reshape(NCORES, -1, p).transpose(0, 2, 1)
                outs.append(blkv)
        return np.ascontiguousarray(np.concatenate(outs, axis=2))

    idx16 = wrap_calls(IDX, 16)                       # [NCORES, 16, TS//16]
    idxw = np.tile(idx16, (1, 8, 1))                  # replicate for 8 Q7 cores
    dstlw = wrap_calls(DSTL, P)                       # [NCORES, 128, TS//128]
    norm1w = wrap_calls(NORM1, P)
    norm2w = wrap_calls(NORM2, P)

    # per-core dinv shards in rid order
    d1s = np.zeros((NCORES, npc), np.float32)
    d2s = np.zeros((NCORES, npc), np.float32)
    valid = orig_of >= 0
    vi = np.where(valid)[0]
    d1pad = np.zeros(NPAD, np.float32)
    d2pad = np.zeros(NPAD, np.float32)
    d1pad[vi] = dinv1[orig_of[vi]]
    d2pad[vi] = dinv2[orig_of[vi]]
    d1s = d1pad.reshape(NCORES, npc)
    d2s = d2pad.reshape(NCORES, npc)

    # permuted x table (fp16 gather table)
    xperm = np.zeros((NPAD, D), np.float16)
    xperm[vi] = np.asarray(x, np.float32)[orig_of[vi]].astype(np.float16)

    # tile/piece schedule (global; identical across cores)
    tiles = []      # (dstl_col, [(k0, k1, rankpos, start, stop)])
    chunk_tiles = []
    last_piece = {}
    for ch in range(nchunks):
        r0, r1 = ch * CHUNK_BLOCKS, min((ch + 1) * CHUNK_BLOCKS, nblocks)
        tlist = []
        for g in range(NG):
            s0 = call_start[ch, g]
            nt = call_slots[ch, g] // P
            bounds = []
            for r in range(r0, r1):
                if padded[r, g]:
                    bounds.append((seg_start[r, g] - s0,
                                   seg_start[r, g] - s0 + padded[r, g], r - r0))
            for t in range(nt):
                t0, t1 = t * P, (t + 1) * P
                wins = sorted({rp for (b0, b1, rp) in bounds
                               if max(t0, b0) < min(t1, b1)})
                tlist.append(((s0 + t0) // P, g, t, wins))
        chunk_tiles.append(tlist)

    pl = Plan()
    pl.N, pl.D, pl.E, pl.R = N, D, E, R
    pl.npc, pl.NPAD, pl.WPAD, pl.nblocks, pl.nchunks, pl.TS = (
        npc, NPAD, WPAD, nblocks, nchunks, TS)
    pl.padded, pl.seg_start = padded, seg_start
    pl.call_slots, pl.call_start = call_slots, call_start
    pl.chunk_tiles = chunk_tiles
    pl.last_piece = last_piece
    pl.idxw, pl.dstlw, pl.norm1w, pl.norm2w = idxw, dstlw, norm1w, norm2w
    pl.d1s, pl.d2s = d1s, d2s
    pl.xperm, pl.orig_of = xperm, orig_of
    pl.rid_of = rid_of
    return pl


def fuzz_consts(c, v, R32):
    """Host constants for one fuzzify stage (rules padded to R32)."""
    R, Dh = c.shape
    iv2 = (1.0 / (v.astype(np.float64) ** 2))
    m = iv2.mean(axis=0)                       # variance-reduction center
    qT = np.zeros((Dh, R32), np.float32)
    qT[:, :R] = (-0.5 * (iv2 - m[None, :])).T
    sT = np.zeros((Dh, R32), np.float32)
    sT[:, :R] = (c.astype(np.float64) * iv2).T
    k = (c.astype(np.float64) ** 2 * iv2).sum(1)     # [R]
    b = -0.5 * k
    C = b.max()
    bias = np.full((R32, 1), -1e30, np.float32)
    bias[:R, 0] = (b - C).astype(np.float32)
    cT = np.zeros((R32, Dh), np.float32)
    cT[:R] = c
    return qT, sT, bias, cT


# ---------------------------------------------------------------------------
# Kernel builder (shared by both layers)
# ---------------------------------------------------------------------------

def build_layer(pl, layer):
    """layer 1: table = x [NPAD,128]; out = table2 shard [npc,64]
       layer 2: table = t2 [NPAD,64]; out = final shard [npc,64]"""
    F_in = pl.D if layer == 1 else 64       # features used per gathered row
    GE = 128                                # gathered row elems (fp16, 256B)
    F_h = 128 if layer == 1 else 64         # fuzzify input dim
    R32 = 32
    npc, TS, nchunks = pl.npc, pl.TS, pl.nchunks
    NDC = CHUNK_BLOCKS * BLK                # dsts per chunk (512)

    nc = bacc.Bacc("TRN2", target_bir_lowering=False, debug=False,
                   num_devices=NCORES)
    dram = lambda n, s, d, k: nc.dram_tensor(n, s, d, kind=k).ap()
    table = dram("table", [pl.NPAD, GE], F16, "ExternalInput")
    IDX = dram("idxs", [P, TS // 16], I16, "ExternalInput")
    DSTL = dram("dstl", [P, TS // P], F32, "ExternalInput")
    NORM = dram("norm", [P, TS // P], F32, "ExternalInput")
    DINVA = dram("dinva", [1, npc], F32, "ExternalInput")
    qT_d = dram("qT", [F_h, R32], F32, "ExternalInput")
    sT_d = dram("sT", [F_h, R32], F32, "ExternalInput")
    bias_d = dram("bias", [R32, 1], F32, "ExternalInput")
    cT_d = dram("cT", [R32, F_h], F32, "ExternalInput")
    if layer == 1:
        W1_d = dram("W1", [pl.D, 128], F32, "ExternalInput")
        B1_d = dram("B1", [128, 1], F32, "ExternalInput")
        W2_d = dram("W2", [128, 64], F32, "ExternalInput")
        DINVB = dram("dinvb", [1, npc], F32, "ExternalInput")
    else:
        B2_d = dram("B2", [64, 1], F32, "ExternalInput")
    OUT = dram("out", [npc, GE] if layer == 1 else [npc, 64],
               F16 if layer == 1 else F32, "ExternalOutput")
    import os
    _dbg = os.environ.get("KDEBUG")
    _stage = os.environ.get("KSTAGE", "full")
    if _dbg:
        DBG = dram("dbg", [F_in, npc], F32, "ExternalOutput")

    nc.gpsimd.load_library(library_config.mlp)

    with tile.TileContext(nc) as tc, ExitStack() as ctx:
        cp = ctx.enter_context(tc.tile_pool(name="const", bufs=1))
        meta = ctx.enter_context(tc.tile_pool(name="meta", bufs=1))
        gp = ctx.enter_context(tc.tile_pool(name="gbuf", bufs=2))
        wp = ctx.enter_context(tc.tile_pool(name="work", bufs=2))
        pp = ctx.enter_context(tc.tile_pool(name="ps", bufs=2, space="PSUM"))
        app = ctx.enter_context(tc.tile_pool(name="aggps", bufs=2, space="PSUM"))
        # PSUM budget (8 banks): agg(2) + mm1(2) + bc(2) + tp(2)

        # ---- constants ----
        # 8 iota tiles, window rp covers chunk-relative dsts [rp*64, rp*64+64)
        iotas = []
        for rp in range(CHUNK_BLOCKS):
            ii = cp.tile([P, BLK], I32, tag=f"ioi{rp}")
            nc.gpsimd.iota(ii[:], pattern=[[1, BLK]], base=rp * BLK,
                           channel_multiplier=0)
            if_ = cp.tile([P, BLK], F16, tag=f"iof{rp}")
            nc.vector.tensor_copy(out=if_[:], in_=ii[:])
            iotas.append(if_)
        ident = cp.tile([P, P], F32)
        from concourse.masks import make_identity
        make_identity(nc, ident[:])
        ones_r = cp.tile([R32, 1], F32)
        nc.vector.memset(ones_r[:], 1.0)
        ones_1 = cp.tile([1, P], F32)
        nc.vector.memset(ones_1[:], 1.0)
        zrow = cp.tile([1, CHUNK_BLOCKS * BLK], F32)
        nc.vector.memset(zrow[:], 0.0)

        qT = cp.tile([F_h, R32], F32); nc.sync.dma_start(out=qT[:], in_=qT_d[:])
        sT = cp.tile([F_h, R32], F32); nc.sync.dma_start(out=sT[:], in_=sT_d[:])
        bias = cp.tile([R32, 1], F32); nc.sync.dma_start(out=bias[:], in_=bias_d[:])
        cT = cp.tile([R32, F_h], F32); nc.sync.dma_start(out=cT[:], in_=cT_d[:])
        if layer == 1:
            W1 = cp.tile([pl.D, 128], F32); nc.sync.dma_start(out=W1[:], in_=W1_d[:])
            B1 = cp.tile([128, 1], F32); nc.sync.dma_start(out=B1[:], in_=B1_d[:])
            W2 = cp.tile([128, 64], F32); nc.sync.dma_start(out=W2[:], in_=W2_d[:])
        else:
            B2 = cp.tile([64, 1], F32); nc.sync.dma_start(out=B2[:], in_=B2_d[:])

        idx_sb = meta.tile([P, TS // 16], I16)
        nc.sync.dma_start(out=idx_sb[:], in_=IDX[:])
        dstl_sb = meta.tile([P, TS // P], F32)
        nc.sync.dma_start(out=dstl_sb[:], in_=DSTL[:])
        norm_sb = meta.tile([P, TS // P], F32)
        nc.sync.dma_start(out=norm_sb[:], in_=NORM[:])

        r32 = lambda ap: ap  # f32r needs rounded producers; plain f32 for now

        for ch in range(nchunks):
            r0 = ch * CHUNK_BLOCKS
            nrk = min(CHUNK_BLOCKS, pl.nblocks - r0)    # ranks in this chunk
            d0 = r0 * BLK
            nd = nrk * BLK                               # dsts this chunk
            # gathers (one per window)
            gbufs = []
            for g in range(NG):
                S = int(pl.call_slots[ch, g])
                gb = gp.tile([P, S // P, GE], F16, tag=f"g{g}")
                nc.gpsimd.dma_gather(
                    out_ap=gb[:],
                    in_ap=table[g * pl.WPAD:(g + 1) * pl.WPAD, :],
                    idxs_ap=idx_sb[:, int(pl.call_start[ch, g]) // 16:
                                   (int(pl.call_start[ch, g]) + S) // 16],
                    num_idxs=S, num_idxs_reg=S, elem_size=GE,
                    single_packet=False,
                )
                gbufs.append(gb)
            # chunk dinv slice
            dinva = wp.tile([1, NDC], F32, tag="dinva")
            nc.sync.dma_start(out=dinva[:1, :nd], in_=DINVA[:1, d0:d0 + nd])
            if layer == 1:
                dinvb = wp.tile([1, NDC], F32, tag="dinvb")
                nc.sync.dma_start(out=dinvb[:1, :nd], in_=DINVB[:1, d0:d0 + nd])

            # ---- segment-sum into one PSUM tile [F_in, nrk*BLK] ----
            # PSUM zero regions are 2KB (whole bank): zero once with a null
            # matmul, then every piece accumulates with start=False.
            agg = app.tile([F_in, NDC], F32, tag="agg", space="PSUM")
            nc.tensor.matmul(out=agg[:, :nd], lhsT=ones_1[:1, :F_in],
                             rhs=zrow[:1, :nd], start=True, stop=False,
                             skip_group_check=True)
            tlist = pl.chunk_tiles[ch]
            npieces = sum(len(wins) for (_, _, _, wins) in tlist)
            pi = 0
            for ti, (dcol, g, t, wins) in enumerate(tlist):
                for rp in wins:
                    pi += 1
                    # T masked to window rp: rows of other windows are zero
                    Tt = wp.tile([P, BLK], F16, tag="Tt")
                    nc.vector.tensor_scalar(
                        out=Tt[:], in0=iotas[rp][:],
                        scalar1=dstl_sb[:, dcol:dcol + 1],
                        scalar2=norm_sb[:, dcol:dcol + 1],
                        op0=mybir.AluOpType.is_equal, op1=mybir.AluOpType.mult)
                    nc.tensor.matmul(
                        out=agg[:, rp * BLK:(rp + 1) * BLK],
                        lhsT=gbufs[g][:, t, :F_in],
                        rhs=Tt[:],
                        start=False,
                        stop=(pi == npieces),
                        skip_group_check=True,
                    )
            acc = wp.tile([F_in, NDC], F32, tag="acc")
            nc.vector.tensor_copy(out=acc[:, :nd], in_=agg[:, :nd])
            if _dbg:
                nc.sync.dma_start(out=DBG[:, d0:d0 + nd], in_=acc[:, :nd])
            if _stage == "agg":
                continue

            # ---- per-chunk transform ----
            # dinvA broadcast [F_in, nd]
            dbc = pp.tile([F_in, NDC], F32, tag="bc", space="PSUM")
            nc.tensor.matmul(out=dbc[:, :nd], lhsT=ones_1[:1, :F_in],
                             rhs=dinva[:1, :nd], start=True, stop=True)
            hv = wp.tile([F_h, NDC], F32, tag="hv")
            if layer == 1:
                hpre = wp.tile([F_in, NDC], F32, tag="hpre")
                nc.vector.tensor_mul(out=hpre[:, :nd], in0=acc[:, :nd],
                                     in1=dbc[:, :nd])
                hps = pp.tile([F_h, NDC], F32, tag="mm1", space="PSUM")
                nc.tensor.matmul(out=hps[:, :nd], lhsT=r32(W1[:]),
                                 rhs=r32(hpre[:, :nd]), start=True, stop=True)
                nc.scalar.activation(out=hv[:, :nd], in_=hps[:, :nd],
                                     func=mybir.ActivationFunctionType.Identity,
                                     bias=B1[:, :1], scale=1.0)
            else:
                nc.vector.tensor_mul(out=hv[:, :nd], in0=acc[:, :nd],
                                     in1=dbc[:, :nd])
                nc.scalar.activation(out=hv[:, :nd], in_=hv[:, :nd],
                                     func=mybir.ActivationFunctionType.Identity,
                                     bias=B2[:, :1], scale=1.0)
            if _stage == "h1":
                continue
            hsq = wp.tile([F_h, NDC], F32, tag="hsq")
            nc.scalar.activation(out=hsq[:, :nd], in_=hv[:, :nd],
                                 func=mybir.ActivationFunctionType.Square)
            zps = pp.tile([R32, NDC], F32, tag="mm1", space="PSUM")
            nc.tensor.matmul(out=zps[:, :nd], lhsT=r32(qT[:]),
                             rhs=r32(hsq[:, :nd]), start=True, stop=False)
            nc.tensor.matmul(out=zps[:, :nd], lhsT=r32(sT[:]),
                             rhs=r32(hv[:, :nd]), start=False, stop=True)
            ez = wp.tile([R32, NDC], F32, tag="ez")
            nc.scalar.activation(out=ez[:, :nd], in_=zps[:, :nd],
                                 func=mybir.ActivationFunctionType.Exp,
                                 bias=bias[:, :1], scale=1.0)
            sps = pp.tile([1, NDC], F32, tag="mm1", space="PSUM")
            nc.tensor.matmul(out=sps[:1, :nd], lhsT=ones_r[:], rhs=ez[:, :nd],
                             start=True, stop=True)
            rcp = wp.tile([1, NDC], F32, tag="rcp")
            nc.vector.reciprocal(out=rcp[:1, :nd], in_=sps[:1, :nd])
            if layer == 1:
                # fold dinv2[d] for the next layer's table
                nc.vector.tensor_mul(out=rcp[:1, :nd], in0=rcp[:1, :nd],
                                     in1=dinvb[:1, :nd])
            ups = pp.tile([F_h, NDC], F32, tag="mm1", space="PSUM")
            nc.tensor.matmul(out=ups[:, :nd], lhsT=r32(cT[:]),
                             rhs=r32(ez[:, :nd]), start=True, stop=True)
            uv = wp.tile([F_h, NDC], F32, tag="uv")
            nc.scalar.activation(out=uv[:, :nd], in_=ups[:, :nd],
                                 func=mybir.ActivationFunctionType.Copy)
            rbc = pp.tile([F_h, NDC], F32, tag="bc", space="PSUM")
            nc.tensor.matmul(out=rbc[:, :nd], lhsT=ones_1[:1, :F_h],
                             rhs=rcp[:1, :nd], start=True, stop=True)
            fz = wp.tile([F_h, NDC], F32, tag="fz")
            nc.vector.tensor_mul(out=fz[:, :nd], in0=uv[:, :nd], in1=rbc[:, :nd])
            if layer == 1:
                t2p = pp.tile([64, NDC], F32, tag="mm1", space="PSUM")
                nc.tensor.matmul(out=t2p[:, :nd], lhsT=r32(W2[:]),
                                 rhs=r32(fz[:, :nd]), start=True, stop=True)
                res = wp.tile([64, NDC], F32, tag="res")
                nc.scalar.activation(out=res[:, :nd], in_=t2p[:, :nd],
                                     func=mybir.ActivationFunctionType.Copy)
            else:
                res = fz
            if _stage == "fz":
                continue
            # ---- transpose to node-major and store ----
            nkt = (nd + P - 1) // P
            for k in range(nkt):
                c0 = k * P
                cw = min(P, nd - c0)
                tp = pp.tile([P, 64], F32, tag="tp", space="PSUM")
                nc.tensor.transpose(out=tp[:cw, :], in_=res[:, c0:c0 + cw],
                                    identity=ident[:64, :64])
                ost = wp.tile([P, 64], F16 if layer == 1 else F32, tag="ost")
                nc.vector.tensor_copy(out=ost[:cw, :], in_=tp[:cw, :])
                nc.sync.dma_start(out=OUT[d0 + c0:d0 + c0 + cw, :64],
                                  in_=ost[:cw, :])

    nc.compile()
    return nc


# ---------------------------------------------------------------------------
# Top-level driver
# ---------------------------------------------------------------------------

_cache = {}
LAST_EXEC_NS = None


def kernel(x, edge_index, edge_weight, W1, b1, W2, b2, c1, v1, c2, v2):
    global LAST_EXEC_NS
    x = np.asarray(x, np.float32)
    R = np.asarray(c1).shape[0]
    pl = preprocess(x, edge_index, edge_weight, R)

    key = (pl.N, pl.E, pl.TS, bytes(pl.padded.tobytes()))
    if key not in _cache:
        _cache[key] = (build_layer(pl, 1), build_layer(pl, 2))
    nc1, nc2 = _cache[key]

    q1, s1, bz1, ct1 = fuzz_consts(np.asarray(c1, np.float32),
                                   np.asarray(v1, np.float32), 32)
    q2, s2, bz2, ct2 = fuzz_consts(np.asarray(c2, np.float32),
                                   np.asarray(v2, np.float32), 32)

    in1 = []
    for c in range(NCORES):
        in1.append({
            "table": pl.xperm,
            "idxs": pl.idxw[c], "dstl": pl.dstlw[c], "norm": pl.norm1w[c],
            "dinva": pl.d1s[c][None, :], "dinvb": pl.d2s[c][None, :],
            "qT": q1, "sT": s1, "bias": bz1, "cT": ct1,
            "W1": np.asarray(W1, np.float32),
            "B1": np.asarray(b1, np.float32)[:, None],
            "W2": np.asarray(W2, np.float32),
        })
    import os as _os
    _tr = bool(_os.environ.get("KTRACE"))
    r1 = run_bass_kernel_spmd(nc1, in1, list(range(NCORES)), trace=_tr)
    t2 = np.concatenate([r1.results[c]["out"] for c in range(NCORES)], axis=0)
    assert t2.dtype == np.float16, t2.dtype

    in2 = []
    for c in range(NCORES):
        in2.append({
            "table": t2,
            "idxs": pl.idxw[c], "dstl": pl.dstlw[c], "norm": pl.norm2w[c],
            "dinva": pl.d2s[c][None, :],
            "qT": q2, "sT": s2, "bias": bz2, "cT": ct2,
            "B2": np.asarray(b2, np.float32)[:, None],
        })
    r2 = run_bass_kernel_spmd(nc2, in2, list(range(NCORES)), trace=_tr)
    outp = np.concatenate([r2.results[c]["out"] for c in range(NCORES)], axis=0)

    if r1.exec_time_ns or r2.exec_time_ns:
        LAST_EXEC_NS = (r1.exec_time_ns or 0) + (r2.exec_time_ns or 0)

    out = np.empty((pl.N, 64), np.float32)
    vi = np.where(pl.orig_of >= 0)[0]
    out[pl.orig_of[vi]] = outp[vi]
    return out


# ---------------------------------------------------------------------------
# Benchmarking: PJRT runner without donation, device-resident inputs.
# ---------------------------------------------------------------------------

def _make_runner(nc):
    import jax
    from jax.experimental.shard_map import shard_map
    from jax.sharding import Mesh, PartitionSpec, NamedSharding
    from concourse import bass2jax, mybir as _mb
    bass2jax.install_neuronx_cc_hook()

    pname = nc.partition_id_tensor.name if nc.partition_id_tensor else None
    in_names, out_names, out_avals, zero_outs = [], [], [], []
    for alloc in nc.m.functions[0].allocations:
        if not isinstance(alloc, _mb.MemoryLocationSet):
            continue
        name = alloc.memorylocations[0].name
        if alloc.kind == "ExternalInput":
            if name != pname:
                in_names.append(name)
        elif alloc.kind == "ExternalOutput":
            out_names.append(name)
            shape = tuple(alloc.tensor_shape)
            dtype = _mb.dt.np(alloc.dtype)
            out_avals.append(jax.core.ShapedArray(shape, dtype))
            zero_outs.append(np.zeros(shape, dtype))
    n_params = len(in_names)
    all_names = in_names + out_names
    if pname is not None:
        all_names = all_names + [pname]

    def _body(*args):
        operands = list(args)
        if pname is not None:
            operands.append(bass2jax.partition_id_tensor())
        outs = bass2jax._bass_exec_p.bind(
            *operands, out_avals=tuple(out_avals), in_names=tuple(all_names),
            out_names=tuple(out_names), lowering_input_output_aliases=(),
            sim_require_finite=True, sim_require_nnan=True, nc=nc)
        return tuple(outs)

    devices = jax.devices()[:NCORES]
    mesh = Mesh(np.asarray(devices), ("core",))
    nio = n_params + len(out_names)
    sharded = jax.jit(
        shard_map(_body, mesh=mesh, in_specs=(PartitionSpec("core"),) * nio,
                  out_specs=(PartitionSpec("core"),) * len(out_names),
                  check_rep=False),
        keep_unused=True)
    sh = NamedSharding(mesh, PartitionSpec("core"))

    def put(in_maps):
        arrs = []
        for i, name in enumerate(in_names):
            cat = np.concatenate([np.asarray(m[name]) for m in in_maps], axis=0)
            arrs.append(jax.device_put(cat, sh))
        for z in zero_outs:
            arrs.append(jax.device_put(
                np.zeros((NCORES * z.shape[0], *z.shape[1:]), z.dtype), sh))
        return arrs

    return sharded, put


def benchmark(x, edge_index, edge_weight, W1, b1, W2, b2, c1, v1, c2, v2,
              iters=10):
    """Returns (t_layer1_s, t_layer2_s, t_empty_s): min wall time per dispatch
    with device-resident inputs; t_empty is the axon dispatch overhead."""
    import jax, time
    x = np.asarray(x, np.float32)
    R = np.asarray(c1).shape[0]
    pl = preprocess(x, edge_index, edge_weight, R)
    key = (pl.N, pl.E, pl.TS, bytes(pl.padded.tobytes()))
    if key not in _cache:
        _cache[key] = (build_layer(pl, 1), build_layer(pl, 2))
    nc1, nc2 = _cache[key]

    q1, s1, bz1, ct1 = fuzz_consts(np.asarray(c1, np.float32),
                                   np.asarray(v1, np.float32), 32)
    q2, s2, bz2, ct2 = fuzz_consts(np.asarray(c2, np.float32),
                                   np.asarray(v2, np.float32), 32)
    in1 = [{"table": pl.xperm, "idxs": pl.idxw[c], "dstl": pl.dstlw[c],
            "norm": pl.norm1w[c], "dinva": pl.d1s[c][None, :],
            "dinvb": pl.d2s[c][None, :], "qT": q1, "sT": s1, "bias": bz1,
            "cT": ct1, "W1": np.asarray(W1, np.float32),
            "B1": np.asarray(b1, np.float32)[:, None],
            "W2": np.asarray(W2, np.float32)} for c in range(NCORES)]
    # layer-2 inputs with a placeholder table (timing only)
    t2 = np.zeros((pl.NPAD, 128), np.float16)
    in2 = [{"table": t2, "idxs": pl.idxw[c], "dstl": pl.dstlw[c],
            "norm": pl.norm2w[c], "dinva": pl.d2s[c][None, :],
            "qT": q2, "sT": s2, "bias": bz2, "cT": ct2,
            "B2": np.asarray(b2, np.float32)[:, None]} for c in range(NCORES)]

    # empty-kernel baseline
    nce = bacc.Bacc("TRN2", target_bir_lowering=False, debug=False,
                    num_devices=NCORES)
    ein = nce.dram_tensor("ein", [128, 16], F32, kind="ExternalInput").ap()
    eout = nce.dram_tensor("eout", [128, 16], F32, kind="ExternalOutput").ap()
    with tile.TileContext(nce) as tc, ExitStack() as ctx:
        sp = ctx.enter_context(tc.tile_pool(name="s", bufs=1))
        t = sp.tile([128, 16], F32)
        nc.sync if False else None
        nce.sync.dma_start(out=t[:], in_=ein[:])
        nce.sync.dma_start(out=eout[:], in_=t[:])
    nce.compile()

    results = []
    for ncx, im in ((nce, [{"ein": np.zeros((128, 16), np.float32)}] * NCORES),
                    (nc1, in1), (nc2, in2)):
        runner, put = _make_runner(ncx)
        args = put(im)
        o = runner(*args); jax.block_until_ready(o)   # warm up / compile
        # async-pipelined: N back-to-back dispatches, block once; the
        # per-dispatch marginal cost approximates device execution time.
        for nbatch in (1, 1 + iters):
            t0 = time.perf_counter()
            outs = [runner(*args) for _ in range(nbatch)]
            jax.block_until_ready(outs)
            if nbatch == 1:
                t_single = time.perf_counter() - t0
            else:
                t_batch = time.perf_counter() - t0
        results.append((t_batch - t_single) / iters)
    t_empty, t1, t2_ = results
    return t1, t2_, t_empty
